# revision 6
# baseline (speedup 1.0000x reference)
"""Trainium2 Bass kernel for nn_MemoryAsContextTitan.

Data-parallel over batch (B=4) on cores 0-3 of 8 (cores 4-7 replicate, their
output is discarded). Per core everything is SBUF-resident; activations are
feature-major [D, tokens] so every linear is a K-tiled matmul with no
transposes. Softmax without max-subtraction (scores provably < 9 here);
MHA denominators via a ones-column in V summed by the av matmul itself,
retrieve denominators via ones-vector matmuls; normalization fused into the
PSUM->SBUF copies. Matmuls float32r; attention probabilities and V are bf16.
The EMA memory update keeps an unscaled running sum (scale 0.9^c folded into
the k/v projection epilogues).

End-to-end wall time is dominated by one-time PJRT session establishment and
host<->device transfer through the tunnel, so: the session warmup starts on a
background thread while the Bass program builds, per-core inputs carry no
redundant broadcast tensors (per-partition bias broadcasts are built on
device with one rank-1 matmul each), and the output travels back as bf16.
"""

import functools
import numpy as np

B, S, D = 4, 3968, 512
H, HD = 8, 64
CHUNK, NPM, MEM = 496, 32, 1024
NCH = S // CHUNK  # 8
KT = D // 128     # 4
MT = MEM // 128   # 8
ISD = float(1.0 / np.float32(np.sqrt(D)))
ISH = float(1.0 / np.float32(np.sqrt(HD)))
QT = [(0, 128), (128, 128), (256, 128), (384, 112)]  # tail q-tiles

WN = ["mq", "mk", "mv"]
WS = ["qp", "aq", "ak", "av"]


def _program():
    import concourse.bass as bass
    import concourse.mybir as mybir
    import concourse.tile as tile
    from concourse import bacc
    from contextlib import ExitStack

    f32 = mybir.dt.float32
    f32r = mybir.dt.float32r
    bf16 = mybir.dt.bfloat16
    Alu = mybir.AluOpType
    Act = mybir.ActivationFunctionType

    def r(ap):
        return ap

    nc = bacc.Bacc("TRN2", target_bir_lowering=False, debug=False)

    xT = nc.dram_tensor("xT", [NCH, D, CHUNK], bf16, kind="ExternalInput").ap()
    pmT = nc.dram_tensor("pmT", [D, NPM], bf16, kind="ExternalInput").ap()
    wd = {n: nc.dram_tensor(f"w_{n}", [D, D], bf16, kind="ExternalInput").ap()
          for n in WN + WS}
    waon_d = nc.dram_tensor("w_aon", [D, D], bf16, kind="ExternalInput").ap()
    bd = {n: nc.dram_tensor(f"b_{n}", [128, KT], f32, kind="ExternalInput").ap()
          for n in ["qp", "mk", "mv", "mq", "aq", "ak", "ao"]}
    brd = {n: nc.dram_tensor(f"br_{n}", [1, D], bf16, kind="ExternalInput").ap()
           for n in ["mv", "av", "ao"]}
    out_d = nc.dram_tensor("out", [NCH, CHUNK, D], bf16, kind="ExternalOutput").ap()

    with nc.allow_low_precision(reason="bf16 attention pipeline, fp32 psum"), \
            tile.TileContext(nc) as tc, ExitStack() as ctx:
        wp = ctx.enter_context(tc.tile_pool(name="wp", bufs=1))
        sp = ctx.enter_context(tc.tile_pool(name="sp", bufs=1))
        ap_ = ctx.enter_context(tc.tile_pool(name="act", bufs=2))
        php = ctx.enter_context(tc.tile_pool(name="php", bufs=3))
        smp = ctx.enter_context(tc.tile_pool(name="smp", bufs=2))
        wsp = ctx.enter_context(tc.tile_pool(name="wsp", bufs=2))
        ps = ctx.enter_context(tc.tile_pool(name="ps", bufs=2, space="PSUM"))

        def wstream(n, c):
            t = wsp.tile([128, KT * D], bf16, name=f"wst_{n}{c}", tag="wstream")
            for kt in range(KT):
                nc.sync.dma_start(out=t[:, kt * D:(kt + 1) * D],
                                  in_=wd[n][kt * 128:(kt + 1) * 128, :])
            return t

        w = {}
        for n in WN:
            w[n] = wp.tile([128, KT * D], bf16, name=f"ws_{n}")
            for kt in range(KT):
                nc.gpsimd.dma_start(out=w[n][:, kt * D:(kt + 1) * D],
                                  in_=wd[n][kt * 128:(kt + 1) * 128, :])
        waon = wp.tile([128, KT * D], bf16, name="ws_aon")
        for kt in range(KT):
            nc.gpsimd.dma_start(out=waon[:, kt * D:(kt + 1) * D],
                                in_=waon_d[kt * 128:(kt + 1) * 128, :])
        bia = {}
        for n in bd:
            bia[n] = wp.tile([128, KT], f32, name=f"bs_{n}")
            nc.gpsimd.dma_start(out=bia[n][:], in_=bd[n][:, :])
        ones_cb = wp.tile([128, 2], bf16, name="ones_cb")
        nc.vector.memset(ones_cb[:], 1.0)
        ones_r = wp.tile([1, 128], bf16, name="ones_r")
        nc.vector.memset(ones_r[:], 1.0)
        one1 = wp.tile([1, 2], bf16, name="one1")
        nc.vector.memset(one1[:], 1.0)
        # per-partition broadcast of the [1, D] bias rows, built on device
        # (rank-1 matmul: ones[1,128]^T @ row[1,D]) instead of shipping
        # 128x duplicated f32 tensors through the tunnel
        bb = {}
        for n in brd:
            br = wp.tile([1, D], bf16, name=f"brs_{n}")
            nc.sync.dma_start(out=br[:], in_=brd[n][:, :])
            bb[n] = wp.tile([128, D], f32, name=f"bbs_{n}")
            pbb = ps.tile([128, 512], f32, name=f"pbb_{n}", tag="proj")
            nc.tensor.matmul(pbb[:], ones_r[0:1, :], br[0:1, :],
                             start=True, stop=True)
            nc.vector.tensor_copy(bb[n][:], pbb[:])

        def wsl(t, kt, dt):
            if isinstance(t, str):
                t = w[t]
            return t[:, kt * D + dt * 128: kt * D + dt * 128 + 128]

        memT = sp.tile([128, KT, MEM], f32, name="memT")
        memB = sp.tile([128, KT, MEM], bf16, name="memB")

        def proj_fm(src, c0, c1, wn, bn, nm):
            """dst[128,KT,T] (feature-major) = W^T @ src[:, :, c0:c1] + b."""
            T = c1 - c0
            dst = ap_.tile([128, KT, T], bf16, name=nm, tag="qry", bufs=3)
            for dt in range(KT):
                p = ps.tile([128, 512], f32, name=f"p_{nm}{dt}", tag="proj")
                for kt in range(KT):
                    nc.tensor.matmul(p[:, 0:T], r(wsl(wn, kt, dt)),
                                     r(src[:, kt, c0:c1]),
                                     start=kt == 0, stop=kt == KT - 1)
                nc.vector.tensor_scalar(dst[:, dt, :], p[:, 0:T],
                                        bia[bn][:, dt:dt + 1], None, Alu.add)
            return dst

        front = {}

        def emit_front(c):
            comb = ap_.tile([128, KT, MEM], bf16, name=f"comb{c}", tag="big")
            if c == 0:
                # mem == 0  =>  hist rows == mv_b exactly
                for dt in range(KT):
                    nc.vector.tensor_scalar(comb[:, dt, NPM:NPM + CHUNK],
                                            bb["mv"][:, 0:CHUNK], 0.0,
                                            bia["mv"][:, dt:dt + 1],
                                            Alu.mult, Alu.add)
            wsq = wstream("qp", c) if c > 0 else None
            for kt in range(KT):
                nc.sync.dma_start(out=comb[:, kt, NPM + CHUNK:MEM],
                                  in_=xT[c, kt * 128:(kt + 1) * 128, :])
                nc.sync.dma_start(out=comb[:, kt, 0:NPM],
                                  in_=pmT[kt * 128:(kt + 1) * 128, :])
            # fused W' = Wq_out @ mq_w (host-precomputed): qp directly from x
            qp = (proj_fm(comb, NPM + CHUNK, MEM, wsq, "qp", f"qpf{c}")
                  if c > 0 else None)
            front[c] = (comb, qp)

        emit_front(0)
        for c in range(NCH):
            smem = 0.9 ** c          # scale of memT entering this chunk
            smem2 = 0.9 ** (c + 1)   # scale after the EMA update
            comb, qp = front.pop(c)

            # ---------------- retrieve 1 -> hist cols of comb ---------------
            if c == 0:
                pass
            else:
                # k/v of retrieve-1 over mem_c are bit-identical to the
                # previous chunk's retrieve-2 projections (same memraw, same
                # folded 0.9^c scale) -- reuse those tiles instead of
                # recomputing 32 matmuls
                kT = prev_k2
                vv = prev_v2
                pavs = [ps.tile([128, 2, 512], f32, name=f"pav{c}{i}",
                                tag="avr", bufs=2) for i in range(2)]
                dn = ps.tile([128, 512], f32, name=f"dn{c}", tag="proj")
                for mt in range(MT):
                    p = ps.tile([128, 512], f32, name=f"psc{c}{mt}", tag="sc")
                    for kt in range(KT):
                        nc.tensor.matmul(p[:, 0:CHUNK],
                                         r(kT[:, kt, mt * 128:mt * 128 + 128]),
                                         r(qp[:, kt, :]),
                                         start=kt == 0, stop=kt == KT - 1)
                    ptm = php.tile([128, MEM], bf16, name=f"pt{c}{mt}",
                                   tag="pth", bufs=6)
                    nc.scalar.activation(ptm[:, 0:CHUNK], p[:, 0:CHUNK],
                                         Act.Exp, scale=ISD)
                    nc.tensor.matmul(dn[0:1, 0:CHUNK], ones_cb[:, 0:1],
                                     ptm[:, 0:CHUNK], start=mt == 0,
                                     stop=mt == MT - 1, skip_group_check=True)
                    for dt in range(KT):
                        nc.tensor.matmul(pavs[dt // 2][:, dt % 2, 0:CHUNK],
                                         vv[:, mt, dt * 128:dt * 128 + 128],
                                         ptm[:, 0:CHUNK], start=mt == 0,
                                         stop=mt == MT - 1,
                                         skip_group_check=True)
                rc = smp.tile([1, 512], bf16, name=f"rc{c}", tag="rc", bufs=1)
                nc.vector.reciprocal(rc[0:1, 0:CHUNK], dn[0:1, 0:CHUNK])
                pb = ps.tile([128, 512], f32, name=f"pb{c}", tag="proj")
                nc.tensor.matmul(pb[:, 0:CHUNK], r(ones_r[0:1, :]),
                                 r(rc[0:1, 0:CHUNK]), start=True, stop=True)
                bcs = smp.tile([128, 512], f32, name=f"bcs{c}", tag="bcs", bufs=1)
                nc.vector.tensor_copy(bcs[:, 0:CHUNK], pb[:, 0:CHUNK])
                for dt in range(KT):
                    nc.vector.tensor_tensor(comb[:, dt, NPM:NPM + CHUNK],
                                            pavs[dt // 2][:, dt % 2, 0:CHUNK],
                                            bcs[:, 0:CHUNK], Alu.mult)

            # ---------------- MHA over combined -----------------------------
            qa = ap_.tile([128, KT, MEM], bf16, name=f"qa{c}", tag="qa", bufs=1)
            ka = ap_.tile([128, KT, MEM], bf16, name=f"ka{c}", tag="kT", bufs=1)
            for dst, wn in ((qa, "aq"), (ka, "ak")):
                wst = wstream(wn, c)
                for dt in range(KT):
                    for hf in range(2):
                        p = ps.tile([128, 512], f32, name=f"p_{wn}{c}{dt}{hf}",
                                    tag="proj")
                        for kt in range(KT):
                            nc.tensor.matmul(
                                p[:], r(wsl(wst, kt, dt)),
                                r(comb[:, kt, hf * 512:hf * 512 + 512]),
                                start=kt == 0, stop=kt == KT - 1)
                        nc.vector.tensor_scalar(
                            dst[:, dt, hf * 512:hf * 512 + 512], p[:],
                            bia[wn][:, dt:dt + 1], None, Alu.add)
            wsv = wstream("av", c)
            va = ap_.tile([128, MT, H, 65], bf16, name=f"va{c}", tag="vv",
                          bufs=1)
            for mt in range(MT):
                p = ps.tile([128, 512], f32, name=f"pva{c}{mt}", tag="proj")
                for kt in range(KT):
                    nc.tensor.matmul(p[:],
                                     r(comb[:, kt, mt * 128:mt * 128 + 128]),
                                     r(wsv[:, kt * D:(kt + 1) * D]),
                                     start=kt == 0, stop=kt == KT - 1)
                nc.vector.tensor_tensor(
                    va[:, mt, :, 0:64],
                    p[:].rearrange("p (h e) -> p h e", h=H),
                    bb["av"][:].rearrange("p (h e) -> p h e", h=H), Alu.add)
            nc.gpsimd.memset(va[:, :, :, 64:65], 1.0)
            if c + 1 < NCH:
                emit_front(c + 1)

            o2 = sp.tile([128, KT, MEM], bf16, name=f"o2{c}", tag="oh", bufs=1)
            for hp2 in range(H // 2):
                # heads 2*hp2 (PE rows 0-63) and 2*hp2+1 (rows 64-127) run
                # concurrently: K=64 matmuls in disjoint row groups
                dth = hp2
                pavr = [ps.tile([128, 2, 512], f32, name=f"pavr{c}{hp2}{i}",
                                tag="avr", bufs=2) for i in range(2)]
                for mt in range(MT):
                    ts = [php.tile([128, MEM], bf16, name=f"pth{c}{hp2}{mt}{e}",
                                   tag="pth", bufs=6) for e in range(2)]
                    for qh in range(2):
                        for e in range(2):
                            hp = e * 64
                            psc = ps.tile([128, 512], f32,
                                          name=f"psa{c}{hp2}{mt}{qh}{e}",
                                          tag="sc")
                            nc.tensor.matmul(
                                psc[:],
                                r(ka[hp:hp + 64, dth, mt * 128:mt * 128 + 128]),
                                r(qa[hp:hp + 64, dth, qh * 512:qh * 512 + 512]),
                                start=True, stop=True)
                            nc.scalar.activation(
                                ts[e][:, qh * 512:qh * 512 + 512],
                                psc[:], Act.Exp, scale=ISH)
                    for e in range(2):
                        h = 2 * hp2 + e
                        for qh in range(2):
                            nc.tensor.matmul(pavr[e][0:65, qh, :],
                                             va[:, mt, h, 0:65],
                                             ts[e][:, qh * 512:qh * 512 + 512],
                                             start=mt == 0, stop=mt == MT - 1,
                                             skip_group_check=True)
                osc = smp.tile([64, MEM], bf16, name=f"osc{c}{hp2}",
                               tag="osc", bufs=2)
                for e in range(2):
                    h = 2 * hp2 + e
                    rch = smp.tile([1, MEM], bf16, name=f"rch{c}{h}", tag="rch",
                                   bufs=2)
                    bch = smp.tile([64, MEM], f32, name=f"bch{c}{h}", tag="bch",
                                   bufs=2)
                    for qh in range(2):
                        pbc = ps.tile([128, 512], f32, name=f"pbc{c}{h}{qh}",
                                      tag="proj")
                        nc.vector.reciprocal(rch[0:1, qh * 512:qh * 512 + 512],
                                             pavr[e][64:65, qh, :])
                        nc.tensor.matmul(pbc[0:64, :], r(ones_r[0:1, 0:64]),
                                         r(rch[0:1, qh * 512:qh * 512 + 512]),
                                         start=True, stop=True)
                        nc.vector.tensor_copy(bch[:, qh * 512:qh * 512 + 512],
                                              pbc[0:64, :])
                        dst = (o2[0:64, hp2, qh * 512:qh * 512 + 512] if e == 0
                               else osc[:, qh * 512:qh * 512 + 512])
                        nc.vector.tensor_tensor(
                            dst, pavr[e][0:64, qh, :],
                            bch[:, qh * 512:qh * 512 + 512], Alu.mult)
                # partition-shift the odd head into rows 64-127
                nc.sync.dma_start(out=o2[64:128, hp2, :], in_=osc[:, :])

            attT = ap_.tile([128, KT, MEM], bf16, name=f"attT{c}", tag="big")
            for dt in range(KT):
                for hf in range(2):
                    p = ps.tile([128, 512], f32, name=f"po{c}{dt}{hf}",
                                tag="proj")
                    for kt in range(KT):
                        nc.tensor.matmul(
                            p[:], wsl(waon, kt, dt),
                            o2[:, kt, hf * 512:hf * 512 + 512],
                            start=kt == 0, stop=kt == KT - 1)
                    nc.vector.tensor_scalar(
                        attT[:, dt, hf * 512:hf * 512 + 512], p[:],
                        bia["ao"][:, dt:dt + 1], None, Alu.add)
            # token-major attended tail rows (for the final elementwise mul)
            ats = []
            for qi, (q0, qn) in enumerate(QT):
                p = ps.tile([128, 512], f32, name=f"pat{c}{qi}", tag="sc")
                for kt in range(KT):
                    nc.tensor.matmul(
                        p[0:qn, :],
                        o2[:, kt, NPM + CHUNK + q0:NPM + CHUNK + q0 + qn],
                        waon[:, kt * D:(kt + 1) * D],
                        start=kt == 0, stop=kt == KT - 1)
                at = smp.tile([128, 512], f32, name=f"at{c}{qi}", tag="at",
                              bufs=4)
                nc.vector.tensor_tensor(at[0:qn, :], p[0:qn, :],
                                        bb["ao"][0:qn, :], Alu.add)
                ats.append(at)

            # ---------------- EMA update (unscaled running sum) --------------
            for dt in range(KT):
                if c == 0:
                    nc.vector.tensor_scalar(memT[:, dt, :], attT[:, dt, :],
                                            0.1 / smem2, None, Alu.mult)
                else:
                    nc.vector.scalar_tensor_tensor(memT[:, dt, :],
                                                   attT[:, dt, :], 0.1 / smem2,
                                                   memT[:, dt, :],
                                                   Alu.mult, Alu.add)

            for dt in range(KT):
                nc.gpsimd.tensor_copy(memB[:, dt, :], memT[:, dt, :])

            # ---------------- retrieve 2 (tail queries only) -----------------
            qp2 = proj_fm(attT, NPM + CHUNK, MEM, "mq", "mq", f"qp2{c}")
            kT2 = ap_.tile([128, KT, MEM], bf16, name=f"kT2{c}", tag="kT",
                           bufs=1)
            for dt in range(KT):
                for hf in range(2):
                    p = ps.tile([128, 512], f32, name=f"pk2{c}{dt}{hf}",
                                tag="proj")
                    for kt in range(KT):
                        nc.tensor.matmul(
                            p[:], r(wsl("mk", kt, dt)),
                            r(memB[:, kt, hf * 512:hf * 512 + 512]),
                            start=kt == 0, stop=kt == KT - 1)
                    nc.vector.tensor_scalar(kT2[:, dt, hf * 512:hf * 512 + 512],
                                            p[:], smem2,
                                            bia["mk"][:, dt:dt + 1],
                                            Alu.mult, Alu.add)
            v2 = ap_.tile([128, MT, 512], bf16, name=f"v2{c}", tag="vv", bufs=1)
            for mt in range(MT):
                p = ps.tile([128, 512], f32, name=f"pv2{c}{mt}", tag="proj")
                for kt in range(KT):
                    nc.tensor.matmul(p[:],
                                     r(memB[:, kt, mt * 128:mt * 128 + 128]),
                                     r(w["mv"][:, kt * D:(kt + 1) * D]),
                                     start=kt == 0, stop=kt == KT - 1)
                nc.vector.scalar_tensor_tensor(v2[:, mt, :], p[:], smem2,
                                               bb["mv"][:], Alu.mult, Alu.add)
            dn2 = ps.tile([128, 512], f32, name=f"dn2{c}", tag="proj")
            pms = [ps.tile([128, 2, 512], f32, name=f"pmo{c}{i}", tag="avr",
                           bufs=2) for i in range(2)]
            for mt in range(MT):
                p = ps.tile([128, 512], f32, name=f"ps2{c}{mt}", tag="sc")
                for kt in range(KT):
                    nc.tensor.matmul(p[:, 0:CHUNK],
                                     r(kT2[:, kt, mt * 128:mt * 128 + 128]),
                                     r(qp2[:, kt, :]),
                                     start=kt == 0, stop=kt == KT - 1)
                ptm = php.tile([128, MEM], bf16, name=f"pt2{c}{mt}",
                               tag="pth", bufs=6)
                nc.scalar.activation(ptm[:, 0:CHUNK], p[:, 0:CHUNK], Act.Exp,
                                     scale=ISD)
                nc.tensor.matmul(dn2[0:1, 0:CHUNK], ones_cb[:, 0:1],
                                 ptm[:, 0:CHUNK], start=mt == 0,
                                 stop=mt == MT - 1, skip_group_check=True)
                for qi, (q0, qn) in enumerate(QT):
                    nc.tensor.matmul(pms[qi // 2][0:qn, qi % 2, :],
                                     ptm[:, q0:q0 + qn],
                                     v2[:, mt, :], start=mt == 0,
                                     stop=mt == MT - 1, skip_group_check=True)
            prev_k2, prev_v2 = kT2, v2
            rc2 = smp.tile([1, 512], bf16, name=f"rc2{c}", tag="rc", bufs=1)
            nc.vector.reciprocal(rc2[0:1, 0:CHUNK], dn2[0:1, 0:CHUNK])
            for qi, (q0, qn) in enumerate(QT):
                prc = ps.tile([128, 512], f32, name=f"prc{c}{qi}", tag="proj")
                nc.tensor.matmul(prc[0:qn, 0:1], r(rc2[0:1, q0:q0 + qn]),
                                 r(one1[0:1, 0:1]), start=True, stop=True)
                rcol = smp.tile([128, 1], f32, name=f"rcol{c}{qi}", tag="rcol",
                                bufs=4)
                nc.vector.tensor_copy(rcol[0:qn, :], prc[0:qn, 0:1])
                ot = smp.tile([128, 512], bf16, name=f"ot{c}{qi}", tag="ot",
                              bufs=4)
                nc.vector.scalar_tensor_tensor(ot[0:qn, :],
                                               pms[qi // 2][0:qn, qi % 2, :],
                                               rcol[0:qn, 0:1],
                                               ats[qi][0:qn, :],
                                               Alu.mult, Alu.mult)
                nc.sync.dma_start(out=out_d[c, q0:q0 + qn, :], in_=ot[0:qn, :])

    nc.compile()
    return nc


@functools.lru_cache(maxsize=1)
def _built():
    return _program()


def _prep_core_inputs(inputs, b):
    import ml_dtypes
    bf = ml_dtypes.bfloat16
    x = np.ascontiguousarray(inputs["x"][b])  # [S, D]
    xT = np.ascontiguousarray(
        x.reshape(NCH, CHUNK, D).transpose(0, 2, 1)).astype(bf)
    im = {"xT": xT,
          "pmT": np.ascontiguousarray(inputs["persistent_memory"].T).astype(bf)}
    wmap = {"mq": "mq_w", "mk": "mk_w", "mv": "mv_w",
            "aq": "aq_w", "ak": "ak_w", "av": "av_w"}
    bmap = {"mq": "mq_b", "mk": "mk_b", "mv": "mv_b",
            "aq": "aq_b", "ak": "ak_b", "ao": "ao_b"}
    for n, src in wmap.items():
        im[f"w_{n}"] = np.ascontiguousarray(inputs[src]).astype(bf)
    w_qp = (inputs["Wq_out"].astype(np.float64)
            @ inputs["mq_w"].astype(np.float64)).astype(np.float32)
    b_qp = (inputs["bq_out"].astype(np.float64)
            @ inputs["mq_w"].astype(np.float64)
            + inputs["mq_b"].astype(np.float64)).astype(np.float32)
    im["w_qp"] = np.ascontiguousarray(w_qp).astype(bf)
    im["b_qp"] = np.ascontiguousarray(b_qp.reshape(KT, 128).T).astype(np.float32)
    im["w_aon"] = np.ascontiguousarray(inputs["ao_w"]).astype(bf)
    for n, src in bmap.items():
        im[f"b_{n}"] = np.ascontiguousarray(
            inputs[src].reshape(KT, 128).T).astype(np.float32)
    for n, src in (("mv", "mv_b"), ("av", "av_b"), ("ao", "ao_b")):
        im[f"br_{n}"] = np.ascontiguousarray(inputs[src][None, :]).astype(bf)
    return im


def _start_session_warmup():
    """PJRT session establishment (first host->device transfer) is the
    dominant one-time cost; kick it off concurrently with the Bass build."""
    import threading

    def _warm():
        try:
            import jax
            for d in jax.devices()[:B]:
                jax.device_put(np.zeros((1, 1), np.float32), d)
        except Exception:
            pass

    t = threading.Thread(target=_warm, daemon=True)
    t.start()
    return t


def kernel(**inputs):
    warm = _start_session_warmup()
    inputs = {k: np.asarray(v) for k, v in inputs.items()}
    nc = _built()
    from concourse.bass_utils import run_bass_kernel_spmd
    in_maps = [_prep_core_inputs(inputs, b) for b in range(B)]
    warm.join()
    import kernel as _k
    res = run_bass_kernel_spmd(nc, in_maps, list(range(B)))
    _k.LAST_RESULTS = res
    out = np.stack([np.asarray(res.results[b]["out"])
                    .astype(np.float32).reshape(S, D)
                    for b in range(B)])
    return out

